# revision 5
# baseline (speedup 1.0000x reference)
"""256-query-chunk variant: finer padded-prefix multiset halves padding waste.

Per core: 8 jobs of 256 queries with padded key prefixes {512, 1024, ...,
4096}. Job jj takes query chunk c in {2jj, 2jj+1} (per-core data decides
which); chunk parity folds into the threshold columns, so the program stays
identical across all 8 cores. Masked tiles = last 4 key tiles of each job.
"""
import contextlib

import numpy as np

import concourse.tile as tile
from concourse import bacc, mybir
from concourse import bass_utils

F32 = mybir.dt.float32
F32R = mybir.dt.float32r
AF = mybir.ActivationFunctionType
ALU = mybir.AluOpType

B, T, C = 4, 4096, 256
N_CORES = 8
QCHUNK = 256
N_JOBS = 8
NQ = QCHUNK * N_JOBS  # 2048 queries per core
CP = C + 2
SCALE = float(C) ** -0.5

_CACHE = {}


def chunks_for(sub):
    """Chunk (of 16 per batch) handled by job jj for core-sub 0/1."""
    out = []
    for k in range(1, 9):          # job jj = k-1, padded prefix 512k
        even_c, odd_c = 2 * k - 2, 2 * k - 1
        if (k % 2 == 1) == (sub == 0):
            out.append(even_c)
        else:
            out.append(odd_c)
    return out


def build(dt_mm=F32R, reps=1):
    nc = bacc.Bacc("TRN2", target_bir_lowering=False, debug=False)

    xkv = nc.dram_tensor("xkv", [C + 1, T], dt_mm, kind="ExternalInput")
    xq = nc.dram_tensor("xq", [C, NQ], dt_mm, kind="ExternalInput")
    wq = nc.dram_tensor("wq", [C, C], dt_mm, kind="ExternalInput")
    wk = nc.dram_tensor("wk", [C, C], dt_mm, kind="ExternalInput")
    wv = nc.dram_tensor("wv", [C + 1, CP], dt_mm, kind="ExternalInput")
    bqk = nc.dram_tensor("bqk", [128, 4], F32, kind="ExternalInput")
    thr = nc.dram_tensor("thr", [128, 32], F32, kind="ExternalInput")
    iot = nc.dram_tensor("iot", [128, QCHUNK], F32, kind="ExternalInput")
    out = nc.dram_tensor("out", [NQ, C], F32, kind="ExternalOutput")

    with tile.TileContext(nc) as tc:
        with contextlib.ExitStack() as ctx:
            consts = ctx.enter_context(tc.tile_pool(name="consts", bufs=1))
            kvp = ctx.enter_context(tc.tile_pool(name="kvp", bufs=1))
            pwork = ctx.enter_context(tc.tile_pool(name="pwork", bufs=4, space="PSUM"))
            pout = ctx.enter_context(tc.tile_pool(name="pout", bufs=4, space="PSUM"))
            expp = ctx.enter_context(tc.tile_pool(name="expp", bufs=4))
            mkp = ctx.enter_context(tc.tile_pool(name="mkp", bufs=2))
            outp = ctx.enter_context(tc.tile_pool(name="outp", bufs=3))
            rcp = ctx.enter_context(tc.tile_pool(name="rcp", bufs=2))

            def body():
                xkv_t = [consts.tile([128, T], dt_mm, tag=f"xkv{i}", name=f"xkv{i}")
                         for i in range(2)]
                ones_t = consts.tile([1, T], dt_mm, tag="ones", name="ones")
                xq_t = [consts.tile([128, NQ], dt_mm, tag=f"xq{i}", name=f"xq{i}")
                        for i in range(2)]
                wq_t = [consts.tile([128, C], dt_mm, tag=f"wq{i}", name=f"wq{i}")
                        for i in range(2)]
                wk_t = [consts.tile([128, C], dt_mm, tag=f"wk{i}", name=f"wk{i}")
                        for i in range(2)]
                wv_t = [consts.tile([128, CP], dt_mm, tag=f"wv{i}", name=f"wv{i}")
                        for i in range(2)]
                wv_ones = consts.tile([1, CP], dt_mm, tag="wvones", name="wvones")
                bqk_t = consts.tile([128, 4], F32, tag="bqk", name="bqk")
                thr_t = consts.tile([128, 32], F32, tag="thr", name="thr")
                iot_t = consts.tile([128, QCHUNK], F32, tag="iot", name="iot")

                for i in range(2):
                    nc.sync.dma_start(out=wq_t[i][:], in_=wq.ap()[128 * i:128 * (i + 1), :])
                for i in range(2):
                    nc.sync.dma_start(out=xq_t[i][:, 0:256],
                                      in_=xq.ap()[128 * i:128 * (i + 1), 0:256])
                nc.sync.dma_start(out=bqk_t[:], in_=bqk.ap()[:])
                for i in range(2):
                    nc.sync.dma_start(out=wk_t[i][:], in_=wk.ap()[128 * i:128 * (i + 1), :])
                for i in range(2):
                    nc.sync.dma_start(out=xkv_t[i][:, 0:512],
                                      in_=xkv.ap()[128 * i:128 * (i + 1), 0:512])
                for i in range(2):
                    nc.sync.dma_start(out=wv_t[i][:], in_=wv.ap()[128 * i:128 * (i + 1), :])
                nc.sync.dma_start(out=wv_ones[:], in_=wv.ap()[C:C + 1, :])
                nc.sync.dma_start(out=ones_t[:], in_=xkv.ap()[C:C + 1, :])
                nc.sync.dma_start(out=thr_t[:], in_=thr.ap()[:])
                nc.sync.dma_start(out=iot_t[:], in_=iot.ap()[:])
                for w in range(1, 8):
                    for i in range(2):
                        nc.sync.dma_start(
                            out=xkv_t[i][:, 512 * w:512 * (w + 1)],
                            in_=xkv.ap()[128 * i:128 * (i + 1), 512 * w:512 * (w + 1)])
                    for i in range(2):
                        nc.sync.dma_start(
                            out=xq_t[i][:, 256 * w:256 * (w + 1)],
                            in_=xq.ap()[128 * i:128 * (i + 1), 256 * w:256 * (w + 1)])

                kT_t = [kvp.tile([128, T], dt_mm, tag=f"kT{i}", name=f"kT{i}")
                        for i in range(2)]
                qT_t = [kvp.tile([128, NQ], dt_mm, tag=f"qT{i}", name=f"qT{i}")
                        for i in range(2)]
                v_t = kvp.tile([128, T // 128, CP], dt_mm, tag="v", name="v")

                def proj_wave(j):
                    """qT cols [256j,+256) first (scores need it now), then kT
                    cols [512j,+512), v tiles 4j..4j+3 (needed late in job)."""
                    for dt_i in range(2):
                        p = pwork.tile([128, 512], F32, tag="pp", name="pp")
                        pq = p[:, 0:QCHUNK]
                        for ct in range(2):
                            nc.tensor.matmul(
                                pq,
                                wq_t[ct][:, 128 * dt_i:128 * (dt_i + 1)],
                                xq_t[ct][:, 256 * j:256 * (j + 1)],
                                start=(ct == 0), stop=(ct == 1),
                            )
                        nc.vector.tensor_scalar(
                            qT_t[dt_i][:, 256 * j:256 * (j + 1)], pq,
                            bqk_t[:, dt_i:dt_i + 1], None, op0=ALU.add,
                        )
                    for dt_i in range(2):
                        p = pwork.tile([128, 512], F32, tag="pp", name="pp")
                        for ct in range(2):
                            nc.tensor.matmul(
                                p[:],
                                wk_t[ct][:, 128 * dt_i:128 * (dt_i + 1)],
                                xkv_t[ct][:, 512 * j:512 * (j + 1)],
                                start=(ct == 0), stop=(ct == 1),
                            )
                        nc.vector.tensor_scalar(
                            kT_t[dt_i][:, 512 * j:512 * (j + 1)], p[:],
                            bqk_t[:, 2 + dt_i:3 + dt_i], None, op0=ALU.add,
                        )
                    for st in range(4 * j, 4 * (j + 1)):
                        p = pwork.tile([128, 512], F32, tag="pp", name="pp")
                        pv = p[:, 0:CP]
                        nc.tensor.matmul(pv, xkv_t[0][:, 128 * st:128 * (st + 1)],
                                         wv_t[0][:], start=True, stop=False)
                        nc.tensor.matmul(pv, xkv_t[1][:, 128 * st:128 * (st + 1)],
                                         wv_t[1][:], start=False, stop=False)
                        nc.tensor.matmul(pv, ones_t[:, 128 * st:128 * (st + 1)],
                                         wv_ones[:], start=False, stop=True)
                        nc.scalar.copy(v_t[:, st, :], pv)

                def emit_scores(j, st):
                    sc = pwork.tile([128, 512], F32, tag="pp", name="pp")[:, 0:QCHUNK]
                    for dt_i in range(2):
                        nc.tensor.matmul(
                            sc,
                            kT_t[dt_i][:, 128 * st:128 * (st + 1)],
                            qT_t[dt_i][:, QCHUNK * j:QCHUNK * (j + 1)],
                            start=(dt_i == 0), stop=(dt_i == 1),
                        )
                    return sc

                mjs = {}

                def emit_masks(j):
                    mj = mkp.tile([128, 4, QCHUNK], F32, tag="mj", name="mj")
                    for q in range(4):
                        nc.vector.tensor_scalar(mj[:, q, :], iot_t[:],
                                                thr_t[:, 4 * j + q:4 * j + q + 1],
                                                None, op0=ALU.is_ge)
                    mjs[j] = mj

                for j in range(N_JOBS):
                    proj_wave(j)
                    if j == 0:
                        emit_masks(0)
                    if j + 1 < N_JOBS:
                        emit_masks(j + 1)
                    mj = mjs[j]
                    n_st = 4 * (j + 1)
                    out_ps = [pout.tile([128, 512], F32, tag="po", name="po")[:, 0:CP]
                              for _ in range(2)]
                    LA = 2
                    scq = [emit_scores(j, k) for k in range(min(LA, n_st))]
                    for st in range(n_st):
                        if st + LA < n_st:
                            scq.append(emit_scores(j, st + LA))
                        sc = scq.pop(0)
                        q_rel = st - (n_st - 4)
                        if q_rel >= 0:
                            et_f = expp.tile([128, QCHUNK], F32, tag="etf", name="etf")
                            nc.scalar.activation(et_f[:], sc, AF.Exp, scale=SCALE)
                            et = expp.tile([128, QCHUNK], dt_mm, tag="et", name="et")
                            nc.vector.tensor_mul(et[:], et_f[:], mj[:, q_rel, :])
                        else:
                            et = expp.tile([128, QCHUNK], dt_mm, tag="et", name="et")
                            nc.scalar.activation(et[:], sc, AF.Exp, scale=SCALE)
                        for m in range(2):
                            nc.tensor.matmul(
                                out_ps[m],
                                et[:, 128 * m:128 * (m + 1)],
                                v_t[:, st, :],
                                start=(st == 0), stop=(st == n_st - 1),
                            )
                    for m in range(2):
                        rc = rcp.tile([128, 1], F32, tag="rc", name="rc")
                        nc.vector.reciprocal(rc[:], out_ps[m][:, C:C + 1])
                        ob = outp.tile([128, C], F32, tag="ob", name="ob")
                        nc.vector.tensor_scalar(ob[:], out_ps[m][:, 0:C], rc[:],
                                                None, op0=ALU.mult)
                        nc.sync.dma_start(
                            out=out.ap()[QCHUNK * j + 128 * m:QCHUNK * j + 128 * (m + 1), :],
                            in_=ob[:],
                        )

            if reps == 1:
                body()
            else:
                with tc.For_i(0, reps, 1):
                    body()
    nc.compile()
    return nc


def _host_prep(x, Wq, bq, Wk, bk, Wv, bv):
    x = np.ascontiguousarray(np.asarray(x, dtype=np.float32))
    Wq, bq = np.asarray(Wq, np.float32), np.asarray(bq, np.float32)
    Wk, bk = np.asarray(Wk, np.float32), np.asarray(bk, np.float32)
    Wv, bv = np.asarray(Wv, np.float32), np.asarray(bv, np.float32)

    wq_h = np.ascontiguousarray(Wq.T)
    wk_h = np.ascontiguousarray(Wk.T)
    wv_h = np.zeros((C + 1, CP), np.float32)
    wv_h[:C, :C] = Wv.T
    wv_h[C, :C] = bv
    wv_h[C, C] = 1.0
    bqk_h = np.stack([bq[0:128], bq[128:256], bk[0:128], bk[128:256]], axis=1)
    iot_h = np.broadcast_to(np.arange(QCHUNK, dtype=np.float32),
                            (128, QCHUNK)).copy()

    p = np.arange(128, dtype=np.float32)[:, None]
    in_maps = []
    for core in range(N_CORES):
        b_i, sub = core // 2, core % 2
        xT = x[b_i].T
        xkv_h = np.concatenate([xT, np.ones((1, T), np.float32)], axis=0)
        chunks = chunks_for(sub)
        xq_h = np.concatenate(
            [xT[:, QCHUNK * c:QCHUNK * (c + 1)] for c in chunks], axis=1)
        thr_h = np.empty((128, 32), np.float32)
        for jj, c in enumerate(chunks):
            for q in range(4):
                thr_h[:, 4 * jj + q] = 128.0 * q + p[:, 0] - 256.0 * (c % 2)
        in_maps.append({
            "xkv": np.ascontiguousarray(xkv_h),
            "xq": np.ascontiguousarray(xq_h),
            "wq": wq_h, "wk": wk_h, "wv": wv_h,
            "bqk": np.ascontiguousarray(bqk_h),
            "thr": thr_h, "iot": iot_h,
        })
    return in_maps


def _gather(results):
    out = np.empty((B, T, C), np.float32)
    for core in range(N_CORES):
        b_i, sub = core // 2, core % 2
        r = results[core]["out"]
        for jj, c in enumerate(chunks_for(sub)):
            out[b_i, QCHUNK * c:QCHUNK * (c + 1), :] = \
                r[QCHUNK * jj:QCHUNK * (jj + 1), :]
    return out


DT_MM = F32R


def _get_nc(reps=1):
    key = (DT_MM, reps)
    if key not in _CACHE:
        _CACHE[key] = build(dt_mm=DT_MM, reps=reps)
    return _CACHE[key]


def kernel(x, Wq, bq, Wk, bk, Wv, bv):
    nc = _get_nc(reps=1)
    in_maps = _host_prep(x, Wq, bq, Wk, bk, Wv, bv)
    res = bass_utils.run_bass_kernel_spmd(nc, in_maps, core_ids=list(range(N_CORES)))
    return _gather(res.results)


# revision 6
# speedup vs baseline: 2.0233x; 2.0233x over previous
"""256-query-chunk variant: finer padded-prefix multiset halves padding waste.

Per core: 8 jobs of 256 queries with padded key prefixes {512, 1024, ...,
4096}. Job jj takes query chunk c in {2jj, 2jj+1} (per-core data decides
which); chunk parity folds into the threshold columns, so the program stays
identical across all 8 cores. Masked tiles = last 4 key tiles of each job.
"""
import contextlib

import numpy as np

import concourse.tile as tile
from concourse import bacc, mybir
from concourse import bass_utils

F32 = mybir.dt.float32
F32R = mybir.dt.float32r
AF = mybir.ActivationFunctionType
ALU = mybir.AluOpType

B, T, C = 4, 4096, 256
N_CORES = 8
QCHUNK = 256
N_JOBS = 8
NQ = QCHUNK * N_JOBS  # 2048 queries per core
CP = C + 2
SCALE = float(C) ** -0.5

_CACHE = {}


def chunks_for(sub):
    """Chunk (of 16 per batch) handled by job jj for core-sub 0/1."""
    out = []
    for k in range(1, 9):          # job jj = k-1, padded prefix 512k
        even_c, odd_c = 2 * k - 2, 2 * k - 1
        if (k % 2 == 1) == (sub == 0):
            out.append(even_c)
        else:
            out.append(odd_c)
    return out


def build(dt_mm=F32R, reps=1):
    nc = bacc.Bacc("TRN2", target_bir_lowering=False, debug=False)

    xkv = nc.dram_tensor("xkv", [C + 1, T], dt_mm, kind="ExternalInput")
    xq = nc.dram_tensor("xq", [C, NQ], dt_mm, kind="ExternalInput")
    wq = nc.dram_tensor("wq", [C, C], dt_mm, kind="ExternalInput")
    wk = nc.dram_tensor("wk", [C, C], dt_mm, kind="ExternalInput")
    wv = nc.dram_tensor("wv", [C + 1, CP], dt_mm, kind="ExternalInput")
    bqk = nc.dram_tensor("bqk", [128, 4], F32, kind="ExternalInput")
    thr = nc.dram_tensor("thr", [128, 32], F32, kind="ExternalInput")
    iot = nc.dram_tensor("iot", [128, QCHUNK], F32, kind="ExternalInput")
    out = nc.dram_tensor("out", [NQ, C], F32, kind="ExternalOutput")

    with tile.TileContext(nc) as tc:
        with contextlib.ExitStack() as ctx:
            consts = ctx.enter_context(tc.tile_pool(name="consts", bufs=1))
            kvp = ctx.enter_context(tc.tile_pool(name="kvp", bufs=1))
            pwork = ctx.enter_context(tc.tile_pool(name="pwork", bufs=5, space="PSUM"))
            pout = ctx.enter_context(tc.tile_pool(name="pout", bufs=3, space="PSUM"))
            expp = ctx.enter_context(tc.tile_pool(name="expp", bufs=4))
            mkp = ctx.enter_context(tc.tile_pool(name="mkp", bufs=2))
            outp = ctx.enter_context(tc.tile_pool(name="outp", bufs=3))
            rcp = ctx.enter_context(tc.tile_pool(name="rcp", bufs=2))

            def body():
                xkv_t = [consts.tile([128, T], dt_mm, tag=f"xkv{i}", name=f"xkv{i}")
                         for i in range(2)]
                ones_t = consts.tile([1, T], dt_mm, tag="ones", name="ones")
                xq_t = [consts.tile([128, NQ], dt_mm, tag=f"xq{i}", name=f"xq{i}")
                        for i in range(2)]
                wq_t = [consts.tile([128, C], dt_mm, tag=f"wq{i}", name=f"wq{i}")
                        for i in range(2)]
                wk_t = [consts.tile([128, C], dt_mm, tag=f"wk{i}", name=f"wk{i}")
                        for i in range(2)]
                wv_t = [consts.tile([128, CP], dt_mm, tag=f"wv{i}", name=f"wv{i}")
                        for i in range(2)]
                wv_ones = consts.tile([1, CP], dt_mm, tag="wvones", name="wvones")
                bqk_t = consts.tile([128, 4], F32, tag="bqk", name="bqk")
                thr_t = consts.tile([128, 32], F32, tag="thr", name="thr")
                iot_t = consts.tile([128, QCHUNK], F32, tag="iot", name="iot")

                for i in range(2):
                    nc.sync.dma_start(out=wq_t[i][:], in_=wq.ap()[128 * i:128 * (i + 1), :])
                for i in range(2):
                    nc.sync.dma_start(out=xq_t[i][:, 0:256],
                                      in_=xq.ap()[128 * i:128 * (i + 1), 0:256])
                nc.sync.dma_start(out=bqk_t[:], in_=bqk.ap()[:])
                for i in range(2):
                    nc.sync.dma_start(out=wk_t[i][:], in_=wk.ap()[128 * i:128 * (i + 1), :])
                for i in range(2):
                    nc.sync.dma_start(out=xkv_t[i][:, 0:512],
                                      in_=xkv.ap()[128 * i:128 * (i + 1), 0:512])
                for i in range(2):
                    nc.sync.dma_start(out=wv_t[i][:], in_=wv.ap()[128 * i:128 * (i + 1), :])
                nc.sync.dma_start(out=wv_ones[:], in_=wv.ap()[C:C + 1, :])
                nc.sync.dma_start(out=ones_t[:], in_=xkv.ap()[C:C + 1, :])
                nc.sync.dma_start(out=thr_t[:], in_=thr.ap()[:])
                nc.sync.dma_start(out=iot_t[:], in_=iot.ap()[:])
                for w in range(1, 8):
                    for i in range(2):
                        nc.sync.dma_start(
                            out=xkv_t[i][:, 512 * w:512 * (w + 1)],
                            in_=xkv.ap()[128 * i:128 * (i + 1), 512 * w:512 * (w + 1)])
                    for i in range(2):
                        nc.sync.dma_start(
                            out=xq_t[i][:, 256 * w:256 * (w + 1)],
                            in_=xq.ap()[128 * i:128 * (i + 1), 256 * w:256 * (w + 1)])

                kT_t = [kvp.tile([128, T], dt_mm, tag=f"kT{i}", name=f"kT{i}")
                        for i in range(2)]
                qT_t = [kvp.tile([128, NQ], dt_mm, tag=f"qT{i}", name=f"qT{i}")
                        for i in range(2)]
                v_t = kvp.tile([128, T // 128, CP], dt_mm, tag="v", name="v")

                def proj_wave(j):
                    """qT cols [256j,+256) first (scores need it now), then kT
                    cols [512j,+512), v tiles 4j..4j+3 (needed late in job)."""
                    for dt_i in range(2):
                        p = pwork.tile([128, 512], F32, tag="pp", name="pp")
                        pq = p[:, 0:QCHUNK]
                        for ct in range(2):
                            nc.tensor.matmul(
                                pq,
                                wq_t[ct][:, 128 * dt_i:128 * (dt_i + 1)],
                                xq_t[ct][:, 256 * j:256 * (j + 1)],
                                start=(ct == 0), stop=(ct == 1),
                            )
                        nc.vector.tensor_scalar(
                            qT_t[dt_i][:, 256 * j:256 * (j + 1)], pq,
                            bqk_t[:, dt_i:dt_i + 1], None, op0=ALU.add,
                        )
                    for dt_i in range(2):
                        p = pwork.tile([128, 512], F32, tag="pp", name="pp")
                        for ct in range(2):
                            nc.tensor.matmul(
                                p[:],
                                wk_t[ct][:, 128 * dt_i:128 * (dt_i + 1)],
                                xkv_t[ct][:, 512 * j:512 * (j + 1)],
                                start=(ct == 0), stop=(ct == 1),
                            )
                        nc.vector.tensor_scalar(
                            kT_t[dt_i][:, 512 * j:512 * (j + 1)], p[:],
                            bqk_t[:, 2 + dt_i:3 + dt_i], None, op0=ALU.add,
                        )
                    for st in range(4 * j, 4 * (j + 1)):
                        p = pwork.tile([128, 512], F32, tag="pp", name="pp")
                        pv = p[:, 0:CP]
                        nc.tensor.matmul(pv, xkv_t[0][:, 128 * st:128 * (st + 1)],
                                         wv_t[0][:], start=True, stop=False)
                        nc.tensor.matmul(pv, xkv_t[1][:, 128 * st:128 * (st + 1)],
                                         wv_t[1][:], start=False, stop=False)
                        nc.tensor.matmul(pv, ones_t[:, 128 * st:128 * (st + 1)],
                                         wv_ones[:], start=False, stop=True)
                        nc.scalar.copy(v_t[:, st, :], pv)

                def emit_scores(j, st):
                    sc = pwork.tile([128, 512], F32, tag="pp", name="pp")[:, 0:QCHUNK]
                    for dt_i in range(2):
                        nc.tensor.matmul(
                            sc,
                            kT_t[dt_i][:, 128 * st:128 * (st + 1)],
                            qT_t[dt_i][:, QCHUNK * j:QCHUNK * (j + 1)],
                            start=(dt_i == 0), stop=(dt_i == 1),
                        )
                    return sc

                mjs = {}

                def emit_masks(j):
                    mj = mkp.tile([128, 4, QCHUNK], F32, tag="mj", name="mj")
                    for q in range(4):
                        nc.vector.tensor_scalar(mj[:, q, :], iot_t[:],
                                                thr_t[:, 4 * j + q:4 * j + q + 1],
                                                None, op0=ALU.is_ge)
                    mjs[j] = mj

                for j in range(N_JOBS):
                    proj_wave(j)
                    if j == 0:
                        emit_masks(0)
                    if j + 1 < N_JOBS:
                        emit_masks(j + 1)
                    mj = mjs[j]
                    n_st = 4 * (j + 1)
                    out_ps = [pout.tile([128, 512], F32, tag="po", name="po")[:, 0:CP]
                              for _ in range(2)]
                    LA = 3
                    scq = [emit_scores(j, k) for k in range(min(LA, n_st))]
                    for st in range(n_st):
                        if st + LA < n_st:
                            scq.append(emit_scores(j, st + LA))
                        sc = scq.pop(0)
                        q_rel = st - (n_st - 4)
                        if q_rel >= 0:
                            et_f = expp.tile([128, QCHUNK], F32, tag="etf", name="etf")
                            nc.scalar.activation(et_f[:], sc, AF.Exp, scale=SCALE)
                            et = expp.tile([128, QCHUNK], dt_mm, tag="et", name="et")
                            nc.vector.tensor_mul(et[:], et_f[:], mj[:, q_rel, :])
                        else:
                            et = expp.tile([128, QCHUNK], dt_mm, tag="et", name="et")
                            nc.scalar.activation(et[:], sc, AF.Exp, scale=SCALE)
                        for m in range(2):
                            nc.tensor.matmul(
                                out_ps[m],
                                et[:, 128 * m:128 * (m + 1)],
                                v_t[:, st, :],
                                start=(st == 0), stop=(st == n_st - 1),
                            )
                    for m in range(2):
                        rc = rcp.tile([128, 1], F32, tag="rc", name="rc")
                        nc.vector.reciprocal(rc[:], out_ps[m][:, C:C + 1])
                        ob = outp.tile([128, C], F32, tag="ob", name="ob")
                        nc.vector.tensor_scalar(ob[:], out_ps[m][:, 0:C], rc[:],
                                                None, op0=ALU.mult)
                        nc.sync.dma_start(
                            out=out.ap()[QCHUNK * j + 128 * m:QCHUNK * j + 128 * (m + 1), :],
                            in_=ob[:],
                        )

            if reps == 1:
                body()
            else:
                with tc.For_i(0, reps, 1):
                    body()
    nc.compile()
    return nc


def _host_prep(x, Wq, bq, Wk, bk, Wv, bv):
    x = np.ascontiguousarray(np.asarray(x, dtype=np.float32))
    Wq, bq = np.asarray(Wq, np.float32), np.asarray(bq, np.float32)
    Wk, bk = np.asarray(Wk, np.float32), np.asarray(bk, np.float32)
    Wv, bv = np.asarray(Wv, np.float32), np.asarray(bv, np.float32)

    wq_h = np.ascontiguousarray(Wq.T)
    wk_h = np.ascontiguousarray(Wk.T)
    wv_h = np.zeros((C + 1, CP), np.float32)
    wv_h[:C, :C] = Wv.T
    wv_h[C, :C] = bv
    wv_h[C, C] = 1.0
    bqk_h = np.stack([bq[0:128], bq[128:256], bk[0:128], bk[128:256]], axis=1)
    iot_h = np.broadcast_to(np.arange(QCHUNK, dtype=np.float32),
                            (128, QCHUNK)).copy()

    p = np.arange(128, dtype=np.float32)[:, None]
    in_maps = []
    for core in range(N_CORES):
        b_i, sub = core // 2, core % 2
        xT = x[b_i].T
        xkv_h = np.concatenate([xT, np.ones((1, T), np.float32)], axis=0)
        chunks = chunks_for(sub)
        xq_h = np.concatenate(
            [xT[:, QCHUNK * c:QCHUNK * (c + 1)] for c in chunks], axis=1)
        thr_h = np.empty((128, 32), np.float32)
        for jj, c in enumerate(chunks):
            for q in range(4):
                thr_h[:, 4 * jj + q] = 128.0 * q + p[:, 0] - 256.0 * (c % 2)
        in_maps.append({
            "xkv": np.ascontiguousarray(xkv_h),
            "xq": np.ascontiguousarray(xq_h),
            "wq": wq_h, "wk": wk_h, "wv": wv_h,
            "bqk": np.ascontiguousarray(bqk_h),
            "thr": thr_h, "iot": iot_h,
        })
    return in_maps


def _gather(results):
    out = np.empty((B, T, C), np.float32)
    for core in range(N_CORES):
        b_i, sub = core // 2, core % 2
        r = results[core]["out"]
        for jj, c in enumerate(chunks_for(sub)):
            out[b_i, QCHUNK * c:QCHUNK * (c + 1), :] = \
                r[QCHUNK * jj:QCHUNK * (jj + 1), :]
    return out


DT_MM = F32R


def _get_nc(reps=1):
    key = (DT_MM, reps)
    if key not in _CACHE:
        _CACHE[key] = build(dt_mm=DT_MM, reps=reps)
    return _CACHE[key]


def kernel(x, Wq, bq, Wk, bk, Wv, bv):
    nc = _get_nc(reps=1)
    in_maps = _host_prep(x, Wq, bq, Wk, bk, Wv, bv)
    res = bass_utils.run_bass_kernel_spmd(nc, in_maps, core_ids=list(range(N_CORES)))
    return _gather(res.results)
